# revision 66
# baseline (speedup 1.0000x reference)
"""Sliding-window attention (BERT-style, window +/-256, RoPE) on 8 TRN2 NeuronCores.

Sharding: core c -> batch b = c//4, head-group g = c%4 (4 of 16 heads each).
Per core: Q/K/V projections in fp16 (scores pre-scaled by folding 8.0 = sqrt(HD)
into Wq on host), RoPE via DMA partition-rotation + DVE muls.

Design (trace-driven, ~145us vs the 172us session-start baseline):
- Score matmuls use K=128 (zero-padded k tiles: k0z rows 0-63 = head0, rows
  64-127 = 0; k1z mirrored; q stays packed) — K=64 matmuls stream ~1.5x
  slower per column.
- Key strips are trimmed per query block (384/512/640 wide); the band mask
  is added with identity matmuls into the score PSUM group (order-free
  accumulates, checks skipped); row-max on DVE, exp on ACT; P is fp16
  end-to-end (the PE pays ~160ns on the first matmul after a dtype/mode
  switch, so v/P/transposes all stay fp16); softmax denominator comes from
  an appended ones-column in V via the PV matmul.
- Attention is a (query-block, head)-granular 3-stage software pipeline:
  scores for unit i+2 and the transpose/evict of unit i are emitted before
  the PV of unit i-1, hiding the DVE max + ACT exp latency and the P^T
  eviction latency from the in-order PE. PSUM "big" ring (3x2 banks) holds
  the live score tiles; one narrow keep-warm matmul per unit keeps the HAM
  clock gate at full rate through the transpose stretch (transpose-mode
  ops don't count as PE activity; at half clock everything runs 2x slow).
- Host pre-arranges xt/w so every input DMA is 128 contiguous rows (serial
  DIRECT2D descriptor programming on the sync engine costs ~1.3ns/row and
  each dma_start lands on a single ~85GB/s queue — loads are chunked
  across queues). k0/q0 projection chunks are interleaved into the V
  projection; their RoPE runs column-halved on the otherwise-idle DVE as
  soon as each half of the raw projection lands (concurrent DVE+GPSIMD big
  ops contend on SBUF, so m=0 rope is all-DVE). The m=1 projection/RoPE is
  scheduled into the m=0 attention stream as PE filler at every other
  unit (denser stealing collapses the PSUM ring pipelining); m=1 rope runs
  on GPSIMD since the DVE is busy with row-maxes there. Dep-free warmup
  matmuls ramp the HAM clock during the initial input-DMA wait.

Self-contained: hardcodes shapes; host side only reshapes/casts/concats.
"""
import os
import sys

sys.path.insert(0, "/opt/trn_rl_repo")

import numpy as np
import ml_dtypes

import concourse.bass as bass
import concourse.mybir as mybir
import concourse.tile as tile
from concourse import bacc
from concourse.bass_utils import run_bass_kernel_spmd

F16 = mybir.dt.float16
BF16 = mybir.dt.bfloat16
F32 = mybir.dt.float32
AF = mybir.ActivationFunctionType
ALU = mybir.AluOpType

B, S, D, H, HD = 2, 2048, 1024, 16, 64
WIN = 256
NSTRIP = 640          # key-strip width per 128-query block
NQB = S // 128        # 16 query blocks
HPC = 4               # heads per core
HDPC = HPC * HD       # 256 output dims per core
ROPE_THETA = 10000.0

LAST_EXEC_NS = None
LAST_RESULTS = None


def strip_start(qb):
    return min(max(qb * 128 - WIN, 0), S - NSTRIP)


MASK_VAL = -60000.0   # fp16-exact large negative, added to scaled scores


def strip_info():
    """Per qb: (sv0, w, segs) where [s0+sv0, s0+sv0+w) is the trimmed key
    strip (cols with no valid row dropped at the edges, 128-aligned) and
    segs is a list of (c0, c1, off) tile-relative mask column runs packed
    into the [128, total] fp16 additive-mask tensor."""
    infos = []
    cols = []
    total = 0
    for qb in range(NQB):
        s0 = strip_start(qb)
        i0 = qb * 128
        ql = np.arange(i0, i0 + 128)[:, None]
        kk = np.arange(s0, s0 + NSTRIP)[None, :]
        valid = (kk >= ql - WIN) & (kk <= ql + WIN)
        anyv = valid.any(axis=0)
        first = int(np.argmax(anyv))
        last = NSTRIP - 1 - int(np.argmax(anyv[::-1]))
        sv0 = (first // 128) * 128
        w = ((last + 1 - sv0 + 127) // 128) * 128
        sub = valid[:, sv0:sv0 + w]
        bad = (~sub).any(axis=0)
        segs = []
        c = 0
        while c < w:
            if bad[c]:
                c1 = c
                while c1 < w and bad[c1]:
                    c1 += 1
                m = np.where(sub[:, c:c1], np.float32(0.0), np.float32(MASK_VAL))
                cols.append(m)
                segs.append((c, c1, total))
                total += c1 - c
                c = c1
            else:
                c += 1
        infos.append((sv0, w, segs))
    packed = np.concatenate(cols, axis=1).astype(np.float16)
    return infos, packed


STRIP_INFOS, MASK_PACKED = strip_info()
MASK_COLS = MASK_PACKED.shape[1]


def rope_tables():
    inv_freq = 1.0 / (ROPE_THETA ** (np.arange(0, HD, 2, dtype=np.float32) / HD))
    t = np.arange(S, dtype=np.float32)
    freqs = np.outer(t, inv_freq)                      # [S, 32]
    emb = np.concatenate([freqs, freqs], axis=-1)      # [S, 64]
    cos = np.cos(emb)                                  # [S, 64]
    sin = np.sin(emb)
    # QT layout [hd-part, s]: partition p uses index p % 64; sign of the
    # rotation term folded into the sin table.
    cosT = np.tile(cos.T, (2, 1))                      # [128, S]
    sinT = np.tile(sin.T, (2, 1))
    sign = np.where((np.arange(128) % 64) < 32, -1.0, 1.0)[:, None]
    return cosT.astype(np.float16), (sinT * sign).astype(np.float16)


_NC_CACHE = None


def build(body_reps=1, ps_bufs=3, share=True, kw_n=64, exp_accum=False,
          gz_vec_memset=False, **_compat):
    nc = bacc.Bacc("TRN2", target_bir_lowering=False, debug=False, num_devices=8)
    # host pre-arranges xt as [p, sl, kt, s'] and w as [p, kt, m] so every
    # load is a 128-row contiguous DMA: ~0.17us of serial DIRECT2D
    # programming per dma_start instead of ~1.4us for the 1024-row layout
    xt_d = nc.dram_tensor("xt", [128, 8 * D // 128 * 256], F16,
                          kind="ExternalInput").ap()
    wq_d = nc.dram_tensor("wq", [128, D // 128 * HDPC], F16,
                          kind="ExternalInput").ap()
    wk_d = nc.dram_tensor("wk", [128, D // 128 * HDPC], F16,
                          kind="ExternalInput").ap()
    wv_d = nc.dram_tensor("wv", [128, D // 128 * HDPC], F16,
                          kind="ExternalInput").ap()
    cos_d = nc.dram_tensor("cosr", [128, S], F16, kind="ExternalInput").ap()
    sin_d = nc.dram_tensor("sinr", [128, S], F16, kind="ExternalInput").ap()
    msk_d = nc.dram_tensor("msk", [128, MASK_COLS], F16, kind="ExternalInput").ap()
    id16_d = nc.dram_tensor("id16", [128, 128], F16, kind="ExternalInput").ap()
    out_d = nc.dram_tensor("out", [S, HDPC], F32, kind="ExternalOutput").ap()

    with tile.TileContext(nc) as tc:
        with (
            tc.tile_pool(name="const", bufs=1) as cpool,
            tc.tile_pool(name="qk", bufs=1) as qkpool,
            tc.tile_pool(name="scratch", bufs=2) as spool,
            tc.tile_pool(name="attn", bufs=3) as apool,
            tc.tile_pool(name="small", bufs=4) as smpool,
            tc.tile_pool(name="ps", bufs=ps_bufs, space="PSUM") as ps,
        ):
            # ---- input loads: every dma_start is 128 contiguous rows ----
            xt_sb = cpool.tile([128, 8, 8, 256], F16, name="xt_sb")
            xt_src = xt_d.rearrange("p (sl kt c) -> p sl kt c", sl=8, kt=8)
            w_sb = {}
            wv_sb = cpool.tile([128, 8, HDPC], F16, name="wv_sb")
            wv_src = wv_d.rearrange("p (kt m) -> p kt m", kt=8)
            w_sb["wv"] = wv_sb

            def xt_slice(sl, h, quarters=False):
                n = 2 if quarters else 4
                for q in range(h * (4 // n), (h + 1) * (4 // n)):
                    nc.sync.dma_start(xt_sb[:, sl, n * q:n * q + n, :],
                                      xt_src[:, sl, n * q:n * q + n, :])

            for q in range(4):
                nc.sync.dma_start(wv_sb[:, 2 * q:2 * q + 2, :],
                                  wv_src[:, 2 * q:2 * q + 2, :])
                nc.sync.dma_start(xt_sb[:, 0, 2 * q:2 * q + 2, :],
                                  xt_src[:, 0, 2 * q:2 * q + 2, :])
            wqk_srcs = {nm: d.rearrange("p (kt m) -> p kt m", kt=8)
                        for nm, d in (("wq", wq_d), ("wk", wk_d))}
            for nm in ("wq", "wk"):
                w_sb[nm] = cpool.tile([128, 8, HDPC], F16, name=nm + "_sb")
            xt_slice(1, 0, quarters=True)
            xt_slice(1, 1, quarters=True)
            for h in range(2):
                nc.sync.dma_start(w_sb["wk"][:, 4 * h:4 * h + 4, :],
                                  wqk_srcs["wk"][:, 4 * h:4 * h + 4, :])
            for sl in range(2, 8):
                xt_slice(sl, 0, quarters=True)
                xt_slice(sl, 1, quarters=True)
                if sl == 2:
                    for h in range(2):
                        nc.sync.dma_start(w_sb["wq"][:, 4 * h:4 * h + 4, :],
                                          wqk_srcs["wq"][:, 4 * h:4 * h + 4, :])
            cos_sb = cpool.tile([128, S], F16, name="cos_sb")
            sin_sb = cpool.tile([128, S], F16, name="sin_sb")
            for h in range(2):
                cs = slice(h * (S // 2), (h + 1) * (S // 2))
                nc.sync.dma_start(cos_sb[:, cs], cos_d[:, cs])
                nc.sync.dma_start(sin_sb[:, cs], sin_d[:, cs])
            id16_sb = cpool.tile([128, 128], F16, name="id16_sb")
            nc.sync.dma_start(id16_sb[:], id16_d)
            msk_sb = cpool.tile([128, MASK_COLS], F16, name="msk_sb")

            # dep-free warmups on a memset tile ramp the HAM clock gate
            # during the initial input-DMA wait (PE is idle there anyway)
            warm_sb = cpool.tile([128, 512], F16, name="warm_sb")
            nc.vector.memset(warm_sb[:], 0.5)
            for dd in range(10):
                wps = ps.tile([128, 512], F32, tag="big", bufs=3,
                              name=f"warmup{dd}")
                nc.tensor.matmul(wps[:], warm_sb[:, 0:128], warm_sb[:],
                                 start=True, stop=True)

            for rep in range(body_reps):
                # ones column appended to v: PV's extra output column yields
                # the softmax denominator for free on the PE (cheaper than
                # the exp accumulator, whose drain costs 182ns of ACT/unit)
                v_sb = cpool.tile([128, NQB, HPC, HD + 1], F16,
                                  tag="v_sb" if share else f"r{rep}v_sb",
                                  name=f"r{rep}v_sb")
                if rep == 0:
                    nc.gpsimd.memset(v_sb[:, :, :, HD:HD + 1], 1.0)

                def v_proj_sb(sb, rep=rep):
                    vps = ps.tile([128, HDPC], F32, tag="big", bufs=ps_bufs,
                                  name=f"r{rep}vps{sb}")
                    for kt in range(8):
                        nc.tensor.matmul(
                            vps[:],
                            xt_sb[:, sb // 2, kt,
                                  (sb % 2) * 128:(sb % 2) * 128 + 128],
                            w_sb["wv"][:, kt, :],
                            start=(kt == 0), stop=(kt == 7))
                    nc.scalar.activation(
                        v_sb[:, sb, :, 0:HD],
                        vps[:].rearrange("p (h c) -> p h c", h=HPC), AF.Copy)

                # k goes into half-zeroed tiles so score matmuls can use K=128
                # (zero rows cancel the other head's q contribution).
                kz = {}
                for m in range(2):
                    for hh in range(2):
                        t = qkpool.tile([128, S], F16,
                                        tag=f"k{hh}z_{m}" if share else f"r{rep}k{hh}z_{m}",
                                        name=f"r{rep}k{hh}z_{m}")
                        if rep == 0:
                            z0 = (1 - hh) * 64
                            eng = nc.vector if gz_vec_memset else nc.gpsimd
                            eng.memset(t[z0:z0 + 64, :], 0.0)
                        kz[(m, hh)] = t

                # ---- Q/K projection + RoPE as schedulable pieces: the m=1
                # pieces are interleaved into the m=0 attention stream so the
                # PE always has dep-free projection work while softmax of the
                # previous unit completes (also keeps the HAM clock warm) ----
                raws = {}
                rots = {}
                t2s = {}
                t1s = {}
                qk_t = {}

                def proj_chunk(nm, m, sc_i, rep=rep):
                    raw = raws.get((nm, m))
                    if raw is None:
                        raw = spool.tile([128, S], F16, tag=f"raw_{nm}{m}",
                                         bufs=1, name=f"r{rep}{nm}raw{m}")
                        raws[(nm, m)] = raw
                    pps = ps.tile([128, 512], F32, tag="big", bufs=ps_bufs,
                                  name=f"r{rep}{nm}ps{m}_{sc_i}")
                    for kt in range(8):
                        nc.tensor.matmul(
                            pps[:],
                            w_sb["w" + nm][:, kt, m * 128:(m + 1) * 128],
                            xt_sb[:, 2 * sc_i:2 * sc_i + 2, kt, :],
                            start=(kt == 0), stop=(kt == 7))
                    nc.scalar.activation(raw[:, sc_i * 512:(sc_i + 1) * 512],
                                         pps[:], AF.Copy)

                def rope_rot(nm, m, ch=slice(0, S), rep=rep):
                    raw = raws[(nm, m)]
                    rot = rots.get((nm, m))
                    if rot is None:
                        rot = spool.tile([128, S], F16, tag=f"rot_{nm}{m}",
                                         bufs=1, name=f"r{rep}{nm}rot{m}")
                        rots[(nm, m)] = rot
                    for gg in range(2):
                        b0 = 64 * gg
                        nc.sync.dma_start(rot[b0:b0 + 32, ch],
                                          raw[b0 + 32:b0 + 64, ch])
                        nc.sync.dma_start(rot[b0 + 32:b0 + 64, ch],
                                          raw[b0:b0 + 32, ch])

                def rope_t2(nm, m, ch=slice(0, S), rep=rep):
                    # m=0 rope on DVE (idle pre-attention; gpsimd is ~4x
                    # slower and concurrent DVE+GPSIMD big ops contend on
                    # SBUF). m=1 rope runs during attention, where the DVE
                    # is loaded with row-maxes — gpsimd takes it despite the
                    # slowness.
                    eng = nc.vector if m == 0 else nc.gpsimd
                    t2 = t2s.get((nm, m))
                    if t2 is None:
                        t2 = spool.tile([128, S], F16, tag="rope_t2",
                                        name=f"r{rep}{nm}t2_{m}")
                        t2s[(nm, m)] = t2
                    eng.tensor_tensor(out=t2[:, ch], in0=rots[(nm, m)][:, ch],
                                      in1=sin_sb[:, ch], op=ALU.mult)

                def rope_rot_t2(nm, m):
                    rope_rot(nm, m)
                    rope_t2(nm, m)

                def rope_t1(nm, m, ch=slice(0, S), rep=rep):
                    eng = nc.vector if m == 0 else nc.gpsimd
                    t1 = t1s.get((nm, m))
                    if t1 is None:
                        t1 = spool.tile([128, S], F16, tag="rope_t1",
                                        name=f"r{rep}{nm}t1_{m}")
                        t1s[(nm, m)] = t1
                    eng.tensor_tensor(out=t1[:, ch], in0=raws[(nm, m)][:, ch],
                                      in1=cos_sb[:, ch], op=ALU.mult)

                ktmps = {}

                def rope_add(nm, m, ch=slice(0, S), rep=rep):
                    eng = nc.vector if m == 0 else nc.gpsimd
                    t1, t2 = t1s[(nm, m)], t2s[(nm, m)]
                    if nm == "q":
                        dst = qk_t.get(("q", m))
                        if dst is None:
                            dst = qkpool.tile([128, S], F16,
                                              tag=f"qk_q_{m}" if share else f"r{rep}qk_q_{m}",
                                              name=f"r{rep}q_sb{m}")
                            qk_t[("q", m)] = dst
                        eng.tensor_tensor(out=dst[:, ch], in0=t1[:, ch],
                                          in1=t2[:, ch], op=ALU.add)
                    else:
                        # full-width add (a [64,*] DVE op runs ~4x slower than
                        # [128,*]), then DMAs scatter the halves into the
                        # zero-padded score operands
                        ktmp = ktmps.get(m)
                        if ktmp is None:
                            ktmp = spool.tile([128, S], F16, tag=f"ktmp{m}",
                                              bufs=1, name=f"r{rep}ktmp{m}")
                            ktmps[m] = ktmp
                        eng.tensor_tensor(out=ktmp[:, ch], in0=t1[:, ch],
                                          in1=t2[:, ch], op=ALU.add)
                        nc.sync.dma_start(kz[(m, 0)][0:64, ch], ktmp[0:64, ch])
                        nc.sync.dma_start(kz[(m, 1)][64:128, ch],
                                          ktmp[64:128, ch])

                # ---- V projection with k0 AND q0 chunks interleaved (chunk
                # i needs the same xt slices as V blocks 4i..4i+3), so both
                # raws finish with the V projection and the whole m=0 RoPE
                # chain runs under the m=1 prefill chunks ----
                # k0 chunks as early as their xt slices allow, q0 after; the
                # k RoPE chain + kz scatter then runs on the DVE right after
                # the V evictions, so k0z/qs0 are ready ~25us before the
                # prefill chunks drain and attention starts immediately.
                H1, H2 = slice(0, S // 2), slice(S // 2, S)

                def rope_half(nm, m, ch):
                    rope_rot(nm, m, ch)
                    rope_t1(nm, m, ch)
                    rope_t2(nm, m, ch)
                    rope_add(nm, m, ch)

                for sb in range(NQB):
                    v_proj_sb(sb)
                    if sb in (3, 4, 5, 6):
                        proj_chunk("k", 0, sb - 3)
                        if sb == 4:
                            rope_half("k", 0, H1)
                        elif sb == 6:
                            rope_half("k", 0, H2)
                    elif sb in (7, 8, 9, 10):
                        proj_chunk("q", 0, sb - 7)
                        if sb == 8:
                            rope_half("q", 0, H1)
                        elif sb == 10:
                            rope_half("q", 0, H2)
                if rep == 0:
                    for c in range(2):
                        cs = slice(c * (MASK_COLS // 2), (c + 1) * (MASK_COLS // 2))
                        nc.sync.dma_start(msk_sb[:, cs], msk_d[:, cs])

                # ---- attention: (qb, head)-granular software pipeline.
                # Score matmuls for unit i+1, i+2 are emitted before the
                # transpose/PV of unit i, so the PE (in-order) always has
                # ~2 units of real work queued while the DVE max + ACT exp
                # of the previous unit complete. "big" bufs=3 holds exactly
                # the 3 live score tiles. ----
                rs_tiles = {}
                ctx_tiles = {}
                pts_tiles = {}
                LOOK = 2

                def emit_scores(m, qb, hh, rep=rep):
                    sv0, w, segs = STRIP_INFOS[qb]
                    k0 = strip_start(qb) + sv0
                    qs = qk_t[("q", m)]
                    scp = ps.tile([128, w], F32, tag="big", bufs=ps_bufs,
                                  name=f"r{rep}sc{m}_{hh}_{qb}")
                    groups = [(0, min(512, w))] + ([(512, w)] if w > 512 else [])
                    for g0, g1 in groups:
                        # score matmul opens AND closes its psum group (full-
                        # region bookkeeping); mask adds accumulate order-free
                        # with checks skipped.
                        nc.tensor.matmul(
                            scp[:, g0:g1],
                            qs[:, qb * 128:(qb + 1) * 128],
                            kz[(m, hh)][:, k0 + g0:k0 + g1],
                            start=True, stop=True)
                        for c0, c1, off in [s for s in segs if g0 <= s[0] < g1]:
                            nc.tensor.matmul(
                                scp[:, c0:c1], id16_sb[:],
                                msk_sb[:, off:off + (c1 - c0)],
                                start=False, stop=False,
                                skip_group_check=True)
                    return scp

                def body_a(m, qb, hh, scp, rep=rep):
                    sv0, w, segs = STRIP_INFOS[qb]
                    kb0 = (strip_start(qb) + sv0) // 128
                    nch = w // 128
                    negmax = smpool.tile([128, 1], F32, tag="negmax",
                                         name=f"r{rep}nm{m}_{hh}_{qb}")
                    nc.vector.tensor_reduce(out=negmax[:], in_=scp[:],
                                            axis=mybir.AxisListType.X,
                                            op=ALU.max, negate=True)
                    p_t = apool.tile([128, w], F16, tag="p",
                                     name=f"r{rep}p{m}_{hh}_{qb}")
                    nc.scalar.activation(p_t[:], scp[:], AF.Exp,
                                         bias=negmax[:], scale=1.0)
                    ptp = ps.tile([128, w], F16, tag="ptps", bufs=1,
                                  name=f"r{rep}ptp{m}_{hh}_{qb}")
                    for j in range(nch):
                        nc.tensor.transpose(ptp[:, j * 128:(j + 1) * 128],
                                            p_t[:, j * 128:(j + 1) * 128],
                                            id16_sb[:])
                    # HAM keep-warm: transposes don't count as PE activity;
                    # one narrow real matmul per unit keeps the duty cycle up
                    # through the transpose/PV stretch.
                    if kw_n:
                        nc.tensor.matmul(scp[:, 0:kw_n], id16_sb[:],
                                         id16_sb[:, 0:kw_n], start=True,
                                         stop=True, skip_group_check=True)
                    pts = apool.tile([128, w], F16, tag="pts",
                                     name=f"r{rep}pts{m}_{hh}_{qb}")
                    if hh == 0:
                        nc.vector.tensor_copy(pts[:], ptp[:])
                    else:
                        nc.scalar.activation(pts[:], ptp[:], AF.Copy)
                    pts_tiles[(m, qb, hh)] = pts

                def body_a2(m, qb, hh, rep=rep):
                    pass

                def body_b(m, qb, hh, rep=rep):
                    sv0, w, segs = STRIP_INFOS[qb]
                    kb0 = (strip_start(qb) + sv0) // 128
                    nch = w // 128
                    pts = pts_tiles.pop((m, qb, hh))
                    if hh == 0:
                        ctx_tiles[(m, qb)] = ps.tile(
                            [128, 2, HD + 1], F32, tag="ctx", bufs=1,
                            name=f"r{rep}ctx{m}_{qb}")
                    ctx = ctx_tiles[(m, qb)]
                    h = 2 * m + hh
                    for j in range(nch):
                        nc.tensor.matmul(ctx[:, hh, :],
                                         pts[:, j * 128:(j + 1) * 128],
                                         v_sb[:, kb0 + j, h, :],
                                         start=(j == 0), stop=(j == nch - 1))
                    if hh == 1:
                        rl = smpool.tile([128, 2, 1], F32, tag="rl",
                                         name=f"r{rep}rl{m}_{qb}")
                        nc.vector.reciprocal(rl[:], ctx[:, :, HD:HD + 1])
                        o_t = smpool.tile([128, 2, HD], F32, tag="o",
                                          name=f"r{rep}o{m}_{qb}")
                        nc.vector.tensor_tensor(
                            out=o_t[:], in0=ctx[:, :, 0:HD],
                            in1=rl[:].broadcast_to([128, 2, HD]),
                            op=ALU.mult)
                        nc.sync.dma_start(
                            out_d[qb * 128:(qb + 1) * 128, m * 128:(m + 1) * 128],
                            o_t[:].rearrange("p a b -> p (a b)"))
                        del ctx_tiles[(m, qb)]

                # m=1 projection/RoPE schedule: 5 chunks before unit 0 cover
                # the m=0 RoPE latency; the rest land at fixed units of the
                # m=0 attention stream as PE filler.
                inserts = {
                    1: [lambda: proj_chunk("q", 1, 0)],
                    3: [lambda: proj_chunk("q", 1, 1)],
                    5: [lambda: proj_chunk("q", 1, 2)],
                    7: [lambda: proj_chunk("q", 1, 3),
                        lambda: rope_rot("q", 1)],
                    9: [lambda: proj_chunk("k", 1, 0),
                        lambda: rope_t1("q", 1)],
                    11: [lambda: proj_chunk("k", 1, 1),
                         lambda: rope_t2("q", 1)],
                    13: [lambda: proj_chunk("k", 1, 2),
                         lambda: rope_add("q", 1)],
                    15: [lambda: proj_chunk("k", 1, 3),
                         lambda: rope_rot("k", 1)],
                    17: [lambda: rope_t1("k", 1)],
                    19: [lambda: rope_t2("k", 1)],
                    21: [lambda: rope_add("k", 1)],
                }

                # staggered two-stage pipeline: unit i's scores emit at
                # iteration i, its softmax/transpose/evict (stage A) at i+2,
                # its PV/store (stage B) at i+3 — the eviction gets a full
                # iteration of slack before the PE needs its result.
                units = [(m, qb, hh) for m in range(2) for qb in range(NQB)
                         for hh in range(2)]
                n_u = len(units)
                live = {}
                for i in range(n_u + 3):
                    if i < n_u:
                        u = units[i]
                        live[u] = emit_scores(*u)
                        for fn in inserts.get(i, ()):
                            fn()
                    if LOOK <= i < n_u + LOOK:
                        v = units[i - LOOK]
                        body_a(*v, live.pop(v))
                    if i >= LOOK + 1:
                        body_b(*units[i - LOOK - 1])
                    if LOOK <= i < n_u + LOOK:
                        body_a2(*units[i - LOOK])
    nc.compile()
    return nc


def make_in_maps(hidden_states, Wq, Wk, Wv):
    cosT, sinT = rope_tables()
    id16 = np.eye(128, dtype=np.float16)

    def xt_layout(xt):
        # [D, S] -> [p, sl*kt*s'] with per-partition contiguous slices
        return np.ascontiguousarray(
            xt.reshape(8, 128, 8, 256).transpose(1, 2, 0, 3).reshape(128, -1))

    def w_layout(wt):
        # [D, HDPC] -> [p, kt*m] per-partition contiguous
        return np.ascontiguousarray(
            wt.reshape(8, 128, HDPC).transpose(1, 0, 2).reshape(128, -1))

    xt16 = [xt_layout(np.asarray(hidden_states, np.float32)[b].T
                      .astype(np.float16)) for b in range(B)]
    in_maps = []
    for c in range(8):
        b, g = c // 4, c % 4
        sl = slice(g * HDPC, (g + 1) * HDPC)
        in_maps.append(dict(
            xt=xt16[b],
            wq=w_layout((np.asarray(Wq, np.float32)[sl, :] * 8.0).T.astype(np.float16)),
            wk=w_layout(np.asarray(Wk, np.float32)[sl, :].T.astype(np.float16)),
            wv=w_layout(np.asarray(Wv, np.float32)[sl, :].T.astype(np.float16)),
            cosr=cosT, sinr=sinT, msk=MASK_PACKED, id16=id16,
        ))
    return in_maps


def kernel(hidden_states, attention_mask, Wq, bq, Wk, bk, Wv, bv):
    global _NC_CACHE, LAST_EXEC_NS, LAST_RESULTS
    attention_mask = np.asarray(attention_mask)
    for bias in (bq, bk, bv):
        assert np.all(np.asarray(bias) == 0.0), "nonzero biases unsupported"

    in_maps = make_in_maps(hidden_states, Wq, Wk, Wv)
    if _NC_CACHE is None:
        _NC_CACHE = build()
    trace = bool(int(os.environ.get("KERNEL_TRACE", "0")))
    res = run_bass_kernel_spmd(_NC_CACHE, in_maps, core_ids=list(range(8)),
                               trace=trace)
    LAST_EXEC_NS = res.exec_time_ns
    LAST_RESULTS = res

    out = np.empty((B, S, D), np.float32)
    for c in range(8):
        b, g = c // 4, c % 4
        out[b, :, g * HDPC:(g + 1) * HDPC] = res.results[c]["out"]
    qmask = (attention_mask > 0).astype(np.float32)[:, :, None]
    return out * qmask


def bench(in_maps, warmup=3, iters=30, nc_override=None):
    """Time repeated executions of the compiled 8-core kernel with inputs
    kept on device. Returns avg seconds per call (upper bound on HW time:
    includes dispatch)."""
    import time
    import jax
    from jax.sharding import Mesh, PartitionSpec
    from jax.experimental.shard_map import shard_map
    from concourse.bass2jax import _bass_exec_p, partition_id_tensor, install_neuronx_cc_hook

    global _NC_CACHE
    if nc_override is not None:
        nc = nc_override
    else:
        if _NC_CACHE is None:
            _NC_CACHE = build()
        nc = _NC_CACHE
    install_neuronx_cc_hook()
    n_cores = 8
    partition_name = nc.partition_id_tensor.name if nc.partition_id_tensor else None
    in_names, out_names, out_avals, zero_outs = [], [], [], []
    for alloc in nc.m.functions[0].allocations:
        if not isinstance(alloc, mybir.MemoryLocationSet):
            continue
        name = alloc.memorylocations[0].name
        if alloc.kind == "ExternalInput":
            if name != partition_name:
                in_names.append(name)
        elif alloc.kind == "ExternalOutput":
            out_names.append(name)
            shape = tuple(alloc.tensor_shape)
            dtype = mybir.dt.np(alloc.dtype)
            out_avals.append(jax.core.ShapedArray(shape, dtype))
            zero_outs.append(np.zeros(shape, dtype))
    n_params = len(in_names)
    n_outs = len(out_avals)
    all_names = in_names + out_names + ([partition_name] if partition_name else [])

    def _body(*args):
        operands = list(args)
        if partition_name is not None:
            operands.append(partition_id_tensor())
        outs = _bass_exec_p.bind(
            *operands, out_avals=tuple(out_avals), in_names=tuple(all_names),
            out_names=tuple(out_names), lowering_input_output_aliases=(),
            sim_require_finite=True, sim_require_nnan=True, nc=nc)
        return tuple(outs)

    devices = jax.devices()[:n_cores]
    mesh = Mesh(np.asarray(devices), ("core",))
    donate = tuple(range(n_params, n_params + n_outs))
    sharded = jax.jit(
        shard_map(_body, mesh=mesh, in_specs=(PartitionSpec("core"),) * (n_params + n_outs),
                  out_specs=(PartitionSpec("core"),) * n_outs, check_rep=False),
        donate_argnums=donate, keep_unused=True)
    concat_in = [np.concatenate([np.asarray(in_maps[c][nm]) for c in range(n_cores)], axis=0)
                 for nm in in_names]
    sharding = jax.sharding.NamedSharding(mesh, PartitionSpec("core"))
    dev_in = [jax.device_put(a, sharding) for a in concat_in]

    def fresh_zeros():
        return [jax.device_put(np.zeros((n_cores * z.shape[0], *z.shape[1:]), z.dtype), sharding)
                for z in zero_outs]

    for _ in range(warmup):
        outs = sharded(*dev_in, *fresh_zeros())
        jax.block_until_ready(outs)
    zsets = [fresh_zeros() for _ in range(iters)]
    jax.block_until_ready(zsets)
    t0 = time.time()
    all_outs = []
    for i in range(iters):
        all_outs.append(sharded(*dev_in, *zsets[i]))
    jax.block_until_ready(all_outs)
    t1 = time.time()
    return (t1 - t0) / iters


def bench_many(in_maps, ncs, warmup=3, iters=40):
    """Interleaved round-robin timing of multiple compiled kernels.
    Returns list of avg seconds per call, drift-robust."""
    import time
    import jax
    from jax.sharding import Mesh, PartitionSpec
    from jax.experimental.shard_map import shard_map
    from concourse.bass2jax import _bass_exec_p, partition_id_tensor, install_neuronx_cc_hook

    install_neuronx_cc_hook()
    n_cores = 8
    devices = jax.devices()[:n_cores]
    mesh = Mesh(np.asarray(devices), ("core",))
    sharding = jax.sharding.NamedSharding(mesh, PartitionSpec("core"))
    entries = []
    for nc in ncs:
        partition_name = nc.partition_id_tensor.name if nc.partition_id_tensor else None
        in_names, out_names, out_avals, zero_outs = [], [], [], []
        for alloc in nc.m.functions[0].allocations:
            if not isinstance(alloc, mybir.MemoryLocationSet):
                continue
            name = alloc.memorylocations[0].name
            if alloc.kind == "ExternalInput":
                if name != partition_name:
                    in_names.append(name)
            elif alloc.kind == "ExternalOutput":
                out_names.append(name)
                shape = tuple(alloc.tensor_shape)
                dtype = mybir.dt.np(alloc.dtype)
                out_avals.append(jax.core.ShapedArray(shape, dtype))
                zero_outs.append(np.zeros(shape, dtype))
        n_params = len(in_names)
        n_outs = len(out_avals)
        all_names = in_names + out_names + ([partition_name] if partition_name else [])

        def _make_body(nc=nc, partition_name=partition_name, out_avals=tuple(out_avals),
                       all_names=tuple(all_names), out_names=tuple(out_names)):
            def _body(*args):
                operands = list(args)
                if partition_name is not None:
                    operands.append(partition_id_tensor())
                return tuple(_bass_exec_p.bind(
                    *operands, out_avals=out_avals, in_names=all_names,
                    out_names=out_names, lowering_input_output_aliases=(),
                    sim_require_finite=True, sim_require_nnan=True, nc=nc))
            return _body

        donate = tuple(range(n_params, n_params + n_outs))
        sharded = jax.jit(
            shard_map(_make_body(), mesh=mesh,
                      in_specs=(PartitionSpec("core"),) * (n_params + n_outs),
                      out_specs=(PartitionSpec("core"),) * n_outs, check_rep=False),
            donate_argnums=donate, keep_unused=True)
        concat_in = [np.concatenate([np.asarray(in_maps[c][nm]) for c in range(n_cores)], axis=0)
                     for nm in in_names]
        dev_in = [jax.device_put(a, sharding) for a in concat_in]

        def fz(zero_outs=zero_outs):
            return [jax.device_put(np.zeros((n_cores * z.shape[0], *z.shape[1:]), z.dtype), sharding)
                    for z in zero_outs]
        entries.append(dict(sharded=sharded, dev_in=dev_in, fz=fz, times=[]))

    chunk = 12
    rounds = max(1, iters // chunk)
    for e in entries:
        for _ in range(warmup):
            jax.block_until_ready(e["sharded"](*e["dev_in"], *e["fz"]()))
    for r in range(rounds):
        for e in entries:
            zsets = [e["fz"]() for _ in range(chunk)]
            jax.block_until_ready(zsets)
            t0 = time.time()
            outs = [e["sharded"](*e["dev_in"], *zsets[i]) for i in range(chunk)]
            jax.block_until_ready(outs)
            e["times"].append((time.time() - t0) / chunk)
    out = []
    for e in entries:
        ts = sorted(e["times"])
        k = max(1, (len(ts) + 1) // 2)
        out.append(sum(ts[:k]) / k)   # mean of fastest half (drift-robust)
    return out
